# revision 21
# baseline (speedup 1.0000x reference)
"""APNB block (sparse pyramid attention) distributed over 8 TRN2 NeuronCores.

Sharding: core c = 2*b + h handles batch item b (of 4) and row-half h (of 2,
60 rows each).  All three 1x1 convs and the attention are data-parallel over
the 7200 local pixels.  The only cross-core data are the pyramid-pooled
key/value sums (collectives over core pairs).

Structure:
  A1: kq = relu(BN(Wk x)) convs (PE+ACT); p6/p8 pooling by DVE column/row
      sums; AllGather#1 of the kq half grids.
  A2: value path, all on PE: per image row, vT[pix, cv] = x^T Wv (row-wise
      matmuls), then pooled via a mask matmul (pool regions are 0/1 columns)
      accumulating [cv, 110] region sums in PSUM; pairwise AllReduce#2 of the
      raw sums (p1/p3 columns are cross-half partial sums).  A2's matmuls
      hide AG1; AR2 hides under the head of phase C.
  C:  per 512-pixel tile, software-pipelined (C2 lags C1 by LAG tiles,
      covering AR2 + the WoV build):
        C1: scoresT = key^T kq (PE) -> exp (ACT) -> colsum via ones-matrix
            matmul (PE) -> 1/x = Exp(-Ln(x)) (ACT) -> attn = exp*recip (GpSimd)
        C2: out = WoV^T attn (PE), bias fused in the PSUM evacuation (ACT/DVE)
      where WoV[s, co] = value[s, :] @ Wo^T is built once per core (the value
      matmul folds into the output conv; Wo@bv folds into the bias).

All matmul operands are bf16 with fp32 PSUM accumulation.
"""

import sys

sys.path.insert(0, "/opt/trn_rl_repo")

import numpy as np

import concourse.bass as bass
import concourse.mybir as mybir
import concourse.tile as tile
from concourse.vector_clock import ScopedClock

F32 = mybir.dt.float32
BF16 = mybir.dt.bfloat16
AF = mybir.ActivationFunctionType
ALU = mybir.AluOpType

N_CORES = 8
B, CIN, H, W = 4, 512, 120, 120
CK, CV, COUT = 256, 256, 512
HL = H // 2          # 60 local rows per core
NPIX = HL * W        # 7200 local pixels
S = 110              # 1 + 9 + 36 + 64 pooled regions
RG = 4               # rows per conv group
NGRP = HL // RG      # 15 conv groups
PIXT = 512           # attention pixel tile
EPS = 1e-5
HALF_GRID = 50       # p6 3x6=18 + p8 4x8=32 per-half regions
LAG = 5              # C2 tiles lag C1 tiles by this much


class SplitDrainTC(tile.TileContext):
    """TileContext whose kernel-tail drain splits its semaphore waits into
    one wait instruction per semaphore (walrus rejects multi-wait
    instructions, and the tail drain otherwise aggregates every live proc)."""

    def _drain_and_barrier(self, tick_clock, wait_clock):
        nc = self.nc
        nc.sync.drain()
        probe = mybir.InstNoOp(
            name="wait-probe", ins=[], outs=[], engine=mybir.EngineType.SP
        )
        wait_clock.add_sem_waits(probe, ScopedClock({None: tick_clock.global_clock}))
        waits = list(probe.sync_info.on_wait or []) if probe.sync_info else []
        name2handle = {
            getattr(h, "name", None): h for h in wait_clock.sems.allocated().values()
        }
        for w in waits:
            h = name2handle.get(w.ant_name)
            assert h is not None, f"no sem handle for {w.ant_name}"
            nc.sync.wait_ge(h, w.wait_value)
        nc.all_engine_barrier()
        popped = nc._tile_sem_poison_stack.pop()
        assert popped is self._sem_poison
        nc.clear_and_free_semaphores(list(self.sems.allocated().values()))
        nc.all_engine_barrier()


def _split_excess_waits(nc):
    """Walrus codegen rejects instructions with more than one sync wait
    (strictest struct: the fused LDWEIGHTS+MATMUL).  Move the excess onto
    EventSemaphore instructions just before the owner on the same engine
    queue, which preserves ordering semantics exactly."""
    for bb in nc.main_func.blocks:
        il = list(bb.instructions)
        out = []
        changed = False
        for ins in il:
            si = ins.sync_info
            waits = list(si.on_wait) if si is not None and si.on_wait else []
            if len(waits) > 1:
                changed = True
                pre, keep = waits[:-1], waits[-1:]
                for j, w in enumerate(pre):
                    ev = mybir.InstEventSemaphore(
                        name=f"wsplit-{ins.name}-{j}",
                        ins=[],
                        outs=[],
                        engine=ins.engine,
                        sync_info=mybir.SyncInfo(on_wait=[w], on_update=[]),
                    )
                    nc.register_instruction(ev)
                    out.append(ev)
                ins.sync_info = mybir.SyncInfo(
                    on_wait=keep, on_update=list(si.on_update or [])
                )
            out.append(ins)
        if changed:
            bb.instructions = out


def build_nc():
    nc = bass.Bass(num_devices=N_CORES)

    x_sh = nc.declare_dram_parameter("x_sh", [CIN, HL, W], BF16, isOutput=False)
    wkT = nc.declare_dram_parameter("wkT", [CIN, CK], BF16, isOutput=False)
    wvT = nc.declare_dram_parameter("wvT", [CIN, CV], BF16, isOutput=False)
    woT = nc.declare_dram_parameter("woT", [CV, COUT], BF16, isOutput=False)
    ones_mat = nc.declare_dram_parameter("ones_mat", [S, S], BF16, isOutput=False)
    bn_inv = nc.declare_dram_parameter("bn_inv", [CK, 1], F32, isOutput=False)
    bn_shift = nc.declare_dram_parameter("bn_shift", [CK, 1], F32, isOutput=False)
    bo_eff = nc.declare_dram_parameter("bo_eff", [COUT, 1], F32, isOutput=False)
    key_crecip = nc.declare_dram_parameter("key_crecip", [128, S], F32, isOutput=False)
    val_crecip = nc.declare_dram_parameter("val_crecip", [128, S], F32, isOutput=False)
    out_d = nc.declare_dram_parameter("out", [COUT, NPIX], F32, isOutput=True)

    with nc.allow_low_precision("bf16 matmul pipeline"), SplitDrainTC(nc) as tc:
        with (
            tc.tile_pool(name="persist", bufs=1) as persist,
            tc.tile_pool(name="dram", bufs=1, space="DRAM") as dram,
        ):
            # ---- constants to SBUF (ACT DMA queue keeps SP free for x) ----
            wk_sb, wv_sb = [], []
            for ki in range(4):
                t = persist.tile([128, CK], BF16, tag=f"wk{ki}", name=f"wk{ki}")
                nc.scalar.dma_start(out=t, in_=wkT[ki * 128 : (ki + 1) * 128, :])
                wk_sb.append(t)
                t = persist.tile([128, CV], BF16, tag=f"wv{ki}", name=f"wv{ki}")
                nc.scalar.dma_start(out=t, in_=wvT[ki * 128 : (ki + 1) * 128, :])
                wv_sb.append(t)
            wo_sb = []
            for mj in range(2):
                t = persist.tile([128, COUT], BF16, tag=f"wo{mj}", name=f"wo{mj}")
                nc.scalar.dma_start(out=t, in_=woT[mj * 128 : (mj + 1) * 128, :])
                wo_sb.append(t)
            bn_inv_sb, bn_shift_sb = [], []
            for mj in range(2):
                t = persist.tile([128, 1], F32, tag=f"bninv{mj}", name=f"bninv{mj}")
                nc.scalar.dma_start(out=t, in_=bn_inv[mj * 128 : (mj + 1) * 128, :])
                bn_inv_sb.append(t)
                t = persist.tile([128, 1], F32, tag=f"bnsh{mj}", name=f"bnsh{mj}")
                nc.scalar.dma_start(out=t, in_=bn_shift[mj * 128 : (mj + 1) * 128, :])
                bn_shift_sb.append(t)
            bo_sb = []
            for co in range(4):
                t = persist.tile([128, 1], F32, tag=f"bo{co}", name=f"bo{co}")
                nc.scalar.dma_start(out=t, in_=bo_eff[co * 128 : (co + 1) * 128, :])
                bo_sb.append(t)
            kcr_sb = persist.tile([128, S], F32, tag="kcr", name="kcr")
            nc.scalar.dma_start(out=kcr_sb, in_=key_crecip[:, :])
            vcr_sb = persist.tile([128, S], F32, tag="vcr", name="vcr")
            nc.scalar.dma_start(out=vcr_sb, in_=val_crecip[:, :])
            ones_sb = persist.tile([S, S], BF16, tag="ones", name="ones")
            nc.scalar.dma_start(out=ones_sb, in_=ones_mat[:, :])

            # ---- persistent working buffers ----
            kq_sb = [
                persist.tile([128, NPIX], BF16, tag=f"kq{mj}", name=f"kq{mj}")
                for mj in range(2)
            ]
            kq_rs5 = [
                persist.tile([128, HL, 24], BF16, tag=f"kqr5{mj}", name=f"kqr5{mj}")
                for mj in range(2)
            ]
            v_rs5 = [
                persist.tile([128, HL, 24], BF16, tag=f"vr5{mj}", name=f"vr5{mj}")
                for mj in range(2)
            ]
            kqp_sb = [
                persist.tile([128, HALF_GRID], F32, tag=f"kqp{mj}", name=f"kqp{mj}")
                for mj in range(2)
            ]
            vp_sb = [
                persist.tile([128, HALF_GRID], F32, tag=f"vp{mj}", name=f"vp{mj}")
                for mj in range(2)
            ]
            key_raw = [
                persist.tile([128, S], F32, tag=f"keyr{mj}", name=f"keyr{mj}")
                for mj in range(2)
            ]
            key_sb = [
                persist.tile([128, S], BF16, tag=f"key{mj}", name=f"key{mj}")
                for mj in range(2)
            ]
            vch_sb = [
                persist.tile([128, S], F32, tag=f"vch{mj}", name=f"vch{mj}")
                for mj in range(2)
            ]
            vchs_sb = [
                persist.tile([128, S], BF16, tag=f"vchs{mj}", name=f"vchs{mj}")
                for mj in range(2)
            ]
            wovT_sb = persist.tile([S, COUT], BF16, tag="wovT", name="wovT")

            ag1_in = dram.tile([2 * 128, HALF_GRID], F32)
            ag1_out = dram.tile([4 * 128, HALF_GRID], F32)
            ag2_in = dram.tile([2 * 128, HALF_GRID], F32)
            ag2_out = dram.tile([4 * 128, HALF_GRID], F32)

            rg = [[0, 1], [2, 3], [4, 5], [6, 7]]

            xall = persist.tile([128, 4, NPIX], BF16, tag="xall", name="xall")
            with (
                tc.tile_pool(name="ps_kq", bufs=2, space="PSUM") as ps_kq,
                tc.tile_pool(name="ps_vt", bufs=2, space="PSUM") as ps_vt,
            ):

                def load_x(g, ng=2):
                    # one strided DMA per ng conv groups into the resident x
                    # copy; rows of a group are contiguous in DRAM (960B+ lines)
                    npix = ng * RG * W
                    nc.sync.dma_start(
                        out=xall[:, :, g * RG * W : g * RG * W + npix],
                        in_=bass.AP(
                            tensor=x_sh[:, :, :].tensor,
                            offset=g * RG * W,
                            ap=[
                                [HL * W, 128],
                                [128 * HL * W, 4],
                                [1, npix],
                            ],
                        ),
                    )

                def stage2(rs5, dst, g):
                    """At the conv groups where a 20-row (p6) or 15-row (p8)
                    block completes, fold its 5-col sums into the half grid
                    with one fused column+row reduce per block."""
                    for mj in range(2):
                        if (g + 1) * RG % 20 == 0:
                            rb = (g + 1) * RG // 20 - 1
                            nc.vector.tensor_reduce(
                                dst[mj][:, rb * 6 : (rb + 1) * 6].rearrange(
                                    "p (a b) -> p a b", a=1
                                ),
                                rs5[mj][:, rb * 20 : (rb + 1) * 20, :].rearrange(
                                    "p r (j f) -> p j r f", f=4
                                ),
                                axis=mybir.AxisListType.XY,
                                op=ALU.add,
                            )
                        for rb in range(4):
                            if g * RG < 15 * (rb + 1) <= (g + 1) * RG:
                                nc.vector.tensor_reduce(
                                    dst[mj][
                                        :, 18 + rb * 8 : 18 + (rb + 1) * 8
                                    ].rearrange("p (a b) -> p a b", a=1),
                                    rs5[mj][:, rb * 15 : (rb + 1) * 15, :].rearrange(
                                        "p r (j f) -> p j r f", f=3
                                    ),
                                    axis=mybir.AxisListType.XY,
                                    op=ALU.add,
                                )

                # ============ Phase A1: kq convs + pooling, AG1 =============
                for g in range(NGRP):
                    load_x(g, ng=1)
                    sl = slice(g * RG * W, (g + 1) * RG * W)
                    for mj in range(2):
                        pk = ps_kq.tile([128, RG * W], F32, tag="pkq", name="pkq")
                        for ki in range(4):
                            nc.tensor.matmul(
                                pk,
                                wk_sb[ki][:, mj * 128 : (mj + 1) * 128],
                                xall[:, ki, sl],
                                start=(ki == 0),
                                stop=(ki == 3),
                            )
                        nc.scalar.activation(
                            kq_sb[mj][:, sl],
                            pk,
                            AF.Relu,
                            bias=bn_shift_sb[mj],
                            scale=bn_inv_sb[mj],
                        )
                        # 5-column sums (the gcd of the 20/15 pooling blocks)
                        nc.vector.tensor_reduce(
                            kq_rs5[mj][:, g * RG : (g + 1) * RG, :],
                            kq_sb[mj][:, sl].rearrange(
                                "p (r c f) -> p r c f", r=RG, c=24
                            ),
                            axis=mybir.AxisListType.X,
                            op=ALU.add,
                        )
                    stage2(kq_rs5, kqp_sb, g)
                for mj in range(2):
                    nc.gpsimd.dma_start(
                        out=ag1_in[mj * 128 : (mj + 1) * 128, :], in_=kqp_sb[mj]
                    )
                nc.gpsimd.collective_compute(
                    "AllGather",
                    ALU.bypass,
                    replica_groups=rg,
                    ins=[ag1_in[:, :].opt()],
                    outs=[ag1_out[:, :].opt()],
                )

                def key_fixup():
                    for mj in range(2):
                        dst = key_raw[mj]
                        r0 = mj * 128
                        r1 = (2 + mj) * 128
                        nc.gpsimd.dma_start(
                            out=dst[:, 10:28], in_=ag1_out[r0 : r0 + 128, 0:18]
                        )
                        nc.gpsimd.dma_start(
                            out=dst[:, 28:46], in_=ag1_out[r1 : r1 + 128, 0:18]
                        )
                        nc.gpsimd.dma_start(
                            out=dst[:, 46:78], in_=ag1_out[r0 : r0 + 128, 18:50]
                        )
                        nc.gpsimd.dma_start(
                            out=dst[:, 78:110], in_=ag1_out[r1 : r1 + 128, 18:50]
                        )
                        nc.vector.tensor_reduce(
                            dst[:, 1:10].rearrange("p (a b) -> p a b", a=3),
                            dst[:, 10:46].rearrange(
                                "p (I di J dj) -> p I J di dj", I=3, di=2, J=3
                            ),
                            axis=mybir.AxisListType.XY,
                            op=ALU.add,
                        )
                        nc.vector.tensor_reduce(
                            dst[:, 0:1],
                            dst[:, 10:46],
                            axis=mybir.AxisListType.X,
                            op=ALU.add,
                        )
                        # sums -> means with the 1/sqrt(ck) score scale folded
                        nc.vector.tensor_mul(key_sb[mj], key_raw[mj], kcr_sb)

                # ==== Phase A2: v convs + DVE pooling from PSUM, AG2 =======
                for g in range(NGRP):
                    sl = slice(g * RG * W, (g + 1) * RG * W)
                    for mj in range(2):
                        pv = ps_vt.tile([128, RG * W], F32, tag="pv", name="pv")
                        for ki in range(4):
                            nc.tensor.matmul(
                                pv,
                                wv_sb[ki][:, mj * 128 : (mj + 1) * 128],
                                xall[:, ki, sl],
                                start=(ki == 0),
                                stop=(ki == 3),
                            )
                        nc.vector.tensor_reduce(
                            v_rs5[mj][:, g * RG : (g + 1) * RG, :],
                            pv.rearrange("p (r c f) -> p r c f", r=RG, c=24),
                            axis=mybir.AxisListType.X,
                            op=ALU.add,
                        )
                    stage2(v_rs5, vp_sb, g)
                    if g == 1:
                        # key fixup rides here: AG1 finished during the early
                        # v groups; emitting it before the AG2 staging avoids
                        # head-of-line blocking on the gpsimd DMA queue.
                        key_fixup()
                for mj in range(2):
                    nc.gpsimd.dma_start(
                        out=ag2_in[mj * 128 : (mj + 1) * 128, :], in_=vp_sb[mj]
                    )
                nc.gpsimd.collective_compute(
                    "AllGather",
                    ALU.bypass,
                    replica_groups=rg,
                    ins=[ag2_in[:, :].opt()],
                    outs=[ag2_out[:, :].opt()],
                )

            # ============ Phase C: attention, software-pipelined ============
            with (
                tc.tile_pool(name="ps_sc", bufs=2, space="PSUM") as ps_sc,
                tc.tile_pool(name="ps_cs", bufs=2, space="PSUM") as ps_cs,
                tc.tile_pool(name="ps_wov", bufs=1, space="PSUM") as ps_wov,
                tc.tile_pool(name="ps_out", bufs=3, space="PSUM") as ps_out,
                tc.tile_pool(name="c1p", bufs=4) as sb_c1,
                tc.tile_pool(name="attn_keep", bufs=LAG + 2) as sb_attn,
                tc.tile_pool(name="outp", bufs=3) as sb_out,
            ):
                offs = list(range(0, NPIX, PIXT))
                attn_tiles = {}

                def emit_c1(t):
                    off = offs[t]
                    N = min(PIXT, NPIX - off)
                    psc = ps_sc.tile([S, PIXT], F32, tag="sc", name="sc")[:, :N]
                    for mj in range(2):
                        nc.tensor.matmul(
                            psc,
                            key_sb[mj],
                            kq_sb[mj][:, off : off + N],
                            start=(mj == 0),
                            stop=(mj == 1),
                        )
                    expt = sb_c1.tile([S, PIXT], BF16, tag="exp", name="exp")[:, :N]
                    nc.scalar.activation(expt, psc, AF.Exp)
                    pcs = ps_cs.tile([S, PIXT], F32, tag="cs", name="cs")[:, :N]
                    nc.tensor.matmul(pcs, ones_sb, expt, start=True, stop=True)
                    lncs = sb_c1.tile([S, PIXT], F32, tag="lncs", name="lncs")[:, :N]
                    nc.scalar.activation(lncs, pcs, AF.Ln)
                    rb = sb_c1.tile([S, PIXT], BF16, tag="rb", name="rb")[:, :N]
                    nc.scalar.activation(rb, lncs, AF.Exp, scale=-1.0)
                    attn = sb_attn.tile([S, PIXT], BF16, tag="attn", name="attn")[
                        :, :N
                    ]
                    nc.gpsimd.tensor_mul(attn, expt, rb)
                    attn_tiles[t] = attn

                def emit_c2(t):
                    off = offs[t]
                    N = min(PIXT, NPIX - off)
                    attn = attn_tiles.pop(t)
                    ot = sb_out.tile([128, 4, PIXT], F32, tag="ot", name="ot")[
                        :, :, :N
                    ]
                    for co in range(4):
                        po = ps_out.tile([128, PIXT], F32, tag="out", name="po")[
                            :, :N
                        ]
                        nc.tensor.matmul(
                            po,
                            wovT_sb[:, co * 128 : (co + 1) * 128],
                            attn,
                            start=True,
                            stop=True,
                        )
                        if co % 2 == 0:
                            nc.scalar.activation(
                                ot[:, co, :], po, AF.Identity, bias=bo_sb[co]
                            )
                        else:
                            nc.vector.tensor_scalar_add(ot[:, co, :], po, bo_sb[co])
                    nc.sync.dma_start(
                        out=bass.AP(
                            tensor=out_d[:, :].tensor,
                            offset=off,
                            ap=[[NPIX, 128], [128 * NPIX, 4], [1, N]],
                        ),
                        in_=ot,
                    )

                for t in range(len(offs)):
                    emit_c1(t)
                    if t == LAG - 1:
                        # value fixup + WoV^T (AG2 has landed by now)
                        for mj in range(2):
                            dst = vch_sb[mj]
                            r0 = mj * 128
                            r1 = (2 + mj) * 128
                            nc.gpsimd.dma_start(
                                out=dst[:, 10:28], in_=ag2_out[r0 : r0 + 128, 0:18]
                            )
                            nc.gpsimd.dma_start(
                                out=dst[:, 28:46], in_=ag2_out[r1 : r1 + 128, 0:18]
                            )
                            nc.gpsimd.dma_start(
                                out=dst[:, 46:78], in_=ag2_out[r0 : r0 + 128, 18:50]
                            )
                            nc.gpsimd.dma_start(
                                out=dst[:, 78:110], in_=ag2_out[r1 : r1 + 128, 18:50]
                            )
                            nc.vector.tensor_reduce(
                                dst[:, 1:10].rearrange("p (a b) -> p a b", a=3),
                                dst[:, 10:46].rearrange(
                                    "p (I di J dj) -> p I J di dj", I=3, di=2, J=3
                                ),
                                axis=mybir.AxisListType.XY,
                                op=ALU.add,
                            )
                            nc.vector.tensor_reduce(
                                dst[:, 0:1],
                                dst[:, 10:46],
                                axis=mybir.AxisListType.X,
                                op=ALU.add,
                            )
                            nc.vector.tensor_mul(vchs_sb[mj], vch_sb[mj], vcr_sb)
                        pwov = ps_wov.tile([S, COUT], F32, tag="wov", name="pwov")
                        for mj in range(2):
                            nc.tensor.matmul(
                                pwov,
                                vchs_sb[mj],
                                wo_sb[mj],
                                start=(mj == 0),
                                stop=(mj == 1),
                            )
                        nc.scalar.activation(wovT_sb, pwov, AF.Copy)
                    if t >= LAG:
                        emit_c2(t - LAG)
                for t in range(len(offs) - LAG, len(offs)):
                    emit_c2(t)
    _split_excess_waits(nc)
    return nc


_CACHE = {}


def _get_nc():
    if "nc" not in _CACHE:
        _CACHE["nc"] = build_nc()
    return _CACHE["nc"]


def _make_masks(h):
    """Per-row pooling masks for local rows of half h: mask[c, r, s] = 1 if
    pixel column c belongs to region s for global row 60*h + r."""
    m = np.zeros((120, HL, S), np.float32)
    for r in range(HL):
        g = h * HL + r
        c = np.arange(120)
        m[:, r, 0] = 1.0
        m[c, r, 1 + (g // 40) * 3 + c // 40] = 1.0
        m[c, r, 10 + (g // 20) * 6 + c // 20] = 1.0
        m[c, r, 46 + (g // 15) * 8 + c // 15] = 1.0
    return m


def kernel(x, Wk, bk, gamma, beta, mean, var, Wv, bv, Wo, bo):
    import ml_dtypes

    from concourse.bass_utils import run_bass_kernel_spmd

    bf16 = ml_dtypes.bfloat16
    x = np.asarray(x, np.float32)
    Wk = np.asarray(Wk, np.float32)
    bk = np.asarray(bk, np.float32)
    gamma = np.asarray(gamma, np.float32)
    beta = np.asarray(beta, np.float32)
    mean = np.asarray(mean, np.float32)
    var = np.asarray(var, np.float32)
    Wv = np.asarray(Wv, np.float32)
    bv = np.asarray(bv, np.float32)
    Wo = np.asarray(Wo, np.float32)
    bo = np.asarray(bo, np.float32)

    inv = gamma / np.sqrt(var + EPS)
    # reference: kq = (Wk x + bk), then BN: kq*inv + (beta - mean*inv)
    shift = beta - mean * inv + bk * inv
    # value = pooled_mean + bv and out = Wo @ value ... : Wo @ bv joins bias
    bo_eff = bo + Wo @ bv

    counts = np.concatenate(
        [
            np.full(1, H * W, np.float32),
            np.full(9, (H // 3) * (W // 3), np.float32),
            np.full(36, (H // 6) * (W // 6), np.float32),
            np.full(64, (H // 8) * (W // 8), np.float32),
        ]
    )
    key_crecip = (
        np.broadcast_to((1.0 / counts)[None, :] * (CK**-0.5), (128, S))
        .astype(np.float32)
        .copy()
    )
    val_crecip = (
        np.broadcast_to((1.0 / counts)[None, :], (128, S)).astype(np.float32).copy()
    )

    common = {
        "wkT": np.ascontiguousarray(Wk.T).astype(bf16),
        "wvT": np.ascontiguousarray(Wv.T).astype(bf16),
        "woT": np.ascontiguousarray(Wo.T).astype(bf16),
        "ones_mat": np.ones((S, S), bf16),
        "bn_inv": inv[:, None].copy(),
        "bn_shift": shift[:, None].copy(),
        "bo_eff": bo_eff[:, None].copy(),
        "key_crecip": key_crecip,
        "val_crecip": val_crecip,
    }
    in_maps = []
    for c in range(N_CORES):
        b, h = c // 2, c % 2
        m = dict(common)
        m["x_sh"] = np.ascontiguousarray(x[b, :, h * HL : (h + 1) * HL, :]).astype(
            bf16
        )
        in_maps.append(m)

    nc = _get_nc()
    _CACHE["last_in_maps"] = in_maps
    res = run_bass_kernel_spmd(nc, in_maps, core_ids=list(range(N_CORES)))
    out = np.empty((B, COUT, H, W), np.float32)
    for c in range(N_CORES):
        b, h = c // 2, c % 2
        out[b, :, h * HL : (h + 1) * HL, :] = res.results[c]["out"].reshape(
            COUT, HL, W
        )
    return out


# revision 22
# speedup vs baseline: 1.1052x; 1.1052x over previous
"""APNB block (sparse pyramid attention) distributed over 8 TRN2 NeuronCores.

Sharding: core c = 2*b + h handles batch item b (of 4) and row-half h (of 2,
60 rows each).  All three 1x1 convs and the attention are data-parallel over
the 7200 local pixels.  The only cross-core data are the pyramid-pooled
key/value sums (collectives over core pairs).

Structure:
  A1: kq = relu(BN(Wk x)) convs (PE+ACT); p6/p8 pooling by DVE column/row
      sums; AllGather#1 of the kq half grids.
  A2: value path, all on PE: per image row, vT[pix, cv] = x^T Wv (row-wise
      matmuls), then pooled via a mask matmul (pool regions are 0/1 columns)
      accumulating [cv, 110] region sums in PSUM; pairwise AllReduce#2 of the
      raw sums (p1/p3 columns are cross-half partial sums).  A2's matmuls
      hide AG1; AR2 hides under the head of phase C.
  C:  per 512-pixel tile, software-pipelined (C2 lags C1 by LAG tiles,
      covering AR2 + the WoV build):
        C1: scoresT = key^T kq (PE) -> exp (ACT) -> colsum via ones-matrix
            matmul (PE) -> 1/x = Exp(-Ln(x)) (ACT) -> attn = exp*recip (GpSimd)
        C2: out = WoV^T attn (PE), bias fused in the PSUM evacuation (ACT/DVE)
      where WoV[s, co] = value[s, :] @ Wo^T is built once per core (the value
      matmul folds into the output conv; Wo@bv folds into the bias).

All matmul operands are bf16 with fp32 PSUM accumulation.
"""

import sys

sys.path.insert(0, "/opt/trn_rl_repo")

import numpy as np

import concourse.bass as bass
import concourse.mybir as mybir
import concourse.tile as tile
from concourse.vector_clock import ScopedClock

F32 = mybir.dt.float32
BF16 = mybir.dt.bfloat16
AF = mybir.ActivationFunctionType
ALU = mybir.AluOpType

N_CORES = 8
B, CIN, H, W = 4, 512, 120, 120
CK, CV, COUT = 256, 256, 512
HL = H // 2          # 60 local rows per core
NPIX = HL * W        # 7200 local pixels
S = 110              # 1 + 9 + 36 + 64 pooled regions
RG = 4               # rows per conv group
NGRP = HL // RG      # 15 conv groups
PIXT = 512           # attention pixel tile
EPS = 1e-5
HALF_GRID = 50       # p6 3x6=18 + p8 4x8=32 per-half regions
LAG = 5              # C2 tiles lag C1 tiles by this much


class SplitDrainTC(tile.TileContext):
    """TileContext whose kernel-tail drain splits its semaphore waits into
    one wait instruction per semaphore (walrus rejects multi-wait
    instructions, and the tail drain otherwise aggregates every live proc)."""

    def _drain_and_barrier(self, tick_clock, wait_clock):
        nc = self.nc
        nc.sync.drain()
        probe = mybir.InstNoOp(
            name="wait-probe", ins=[], outs=[], engine=mybir.EngineType.SP
        )
        wait_clock.add_sem_waits(probe, ScopedClock({None: tick_clock.global_clock}))
        waits = list(probe.sync_info.on_wait or []) if probe.sync_info else []
        name2handle = {
            getattr(h, "name", None): h for h in wait_clock.sems.allocated().values()
        }
        for w in waits:
            h = name2handle.get(w.ant_name)
            assert h is not None, f"no sem handle for {w.ant_name}"
            nc.sync.wait_ge(h, w.wait_value)
        nc.all_engine_barrier()
        popped = nc._tile_sem_poison_stack.pop()
        assert popped is self._sem_poison
        nc.clear_and_free_semaphores(list(self.sems.allocated().values()))
        nc.all_engine_barrier()


def _split_excess_waits(nc):
    """Walrus codegen rejects instructions with more than one sync wait
    (strictest struct: the fused LDWEIGHTS+MATMUL).  Move the excess onto
    EventSemaphore instructions just before the owner on the same engine
    queue, which preserves ordering semantics exactly."""
    for bb in nc.main_func.blocks:
        il = list(bb.instructions)
        out = []
        changed = False
        for ins in il:
            si = ins.sync_info
            waits = list(si.on_wait) if si is not None and si.on_wait else []
            if len(waits) > 1:
                changed = True
                pre, keep = waits[:-1], waits[-1:]
                for j, w in enumerate(pre):
                    ev = mybir.InstEventSemaphore(
                        name=f"wsplit-{ins.name}-{j}",
                        ins=[],
                        outs=[],
                        engine=ins.engine,
                        sync_info=mybir.SyncInfo(on_wait=[w], on_update=[]),
                    )
                    nc.register_instruction(ev)
                    out.append(ev)
                ins.sync_info = mybir.SyncInfo(
                    on_wait=keep, on_update=list(si.on_update or [])
                )
            out.append(ins)
        if changed:
            bb.instructions = out


def build_nc():
    nc = bass.Bass(num_devices=N_CORES)

    x_sh = nc.declare_dram_parameter("x_sh", [CIN, HL, W], BF16, isOutput=False)
    wkT = nc.declare_dram_parameter("wkT", [CIN, CK], BF16, isOutput=False)
    wvT = nc.declare_dram_parameter("wvT", [CIN, CV], BF16, isOutput=False)
    woT = nc.declare_dram_parameter("woT", [CV, COUT], BF16, isOutput=False)
    ones_mat = nc.declare_dram_parameter("ones_mat", [S, S], BF16, isOutput=False)
    bn_inv = nc.declare_dram_parameter("bn_inv", [CK, 1], F32, isOutput=False)
    bn_shift = nc.declare_dram_parameter("bn_shift", [CK, 1], F32, isOutput=False)
    bo_eff = nc.declare_dram_parameter("bo_eff", [COUT, 1], F32, isOutput=False)
    key_crecip = nc.declare_dram_parameter("key_crecip", [128, S], F32, isOutput=False)
    val_crecip = nc.declare_dram_parameter("val_crecip", [128, S], F32, isOutput=False)
    out_d = nc.declare_dram_parameter("out", [COUT, NPIX], F32, isOutput=True)

    with nc.allow_low_precision("bf16 matmul pipeline"), SplitDrainTC(nc) as tc:
        with (
            tc.tile_pool(name="persist", bufs=1) as persist,
            tc.tile_pool(name="dram", bufs=1, space="DRAM") as dram,
        ):
            # ---- constants to SBUF (ACT DMA queue keeps SP free for x) ----
            wk_sb, wv_sb = [], []
            for ki in range(4):
                t = persist.tile([128, CK], BF16, tag=f"wk{ki}", name=f"wk{ki}")
                nc.scalar.dma_start(out=t, in_=wkT[ki * 128 : (ki + 1) * 128, :])
                wk_sb.append(t)
                t = persist.tile([128, CV], BF16, tag=f"wv{ki}", name=f"wv{ki}")
                nc.scalar.dma_start(out=t, in_=wvT[ki * 128 : (ki + 1) * 128, :])
                wv_sb.append(t)
            wo_sb = []
            for mj in range(2):
                t = persist.tile([128, COUT], BF16, tag=f"wo{mj}", name=f"wo{mj}")
                nc.scalar.dma_start(out=t, in_=woT[mj * 128 : (mj + 1) * 128, :])
                wo_sb.append(t)
            bn_inv_sb, bn_shift_sb = [], []
            for mj in range(2):
                t = persist.tile([128, 1], F32, tag=f"bninv{mj}", name=f"bninv{mj}")
                nc.scalar.dma_start(out=t, in_=bn_inv[mj * 128 : (mj + 1) * 128, :])
                bn_inv_sb.append(t)
                t = persist.tile([128, 1], F32, tag=f"bnsh{mj}", name=f"bnsh{mj}")
                nc.scalar.dma_start(out=t, in_=bn_shift[mj * 128 : (mj + 1) * 128, :])
                bn_shift_sb.append(t)
            bo_sb = []
            for co in range(4):
                t = persist.tile([128, 1], F32, tag=f"bo{co}", name=f"bo{co}")
                nc.scalar.dma_start(out=t, in_=bo_eff[co * 128 : (co + 1) * 128, :])
                bo_sb.append(t)
            kcr_sb = persist.tile([128, S], F32, tag="kcr", name="kcr")
            nc.scalar.dma_start(out=kcr_sb, in_=key_crecip[:, :])
            vcr_sb = persist.tile([128, S], F32, tag="vcr", name="vcr")
            nc.scalar.dma_start(out=vcr_sb, in_=val_crecip[:, :])
            ones_sb = persist.tile([S, S], BF16, tag="ones", name="ones")
            nc.scalar.dma_start(out=ones_sb, in_=ones_mat[:, :])

            # ---- persistent working buffers ----
            kq_sb = [
                persist.tile([128, NPIX], BF16, tag=f"kq{mj}", name=f"kq{mj}")
                for mj in range(2)
            ]
            kq_rs5 = [
                persist.tile([128, HL, 24], BF16, tag=f"kqr5{mj}", name=f"kqr5{mj}")
                for mj in range(2)
            ]
            v_rs5 = [
                persist.tile([128, HL, 24], BF16, tag=f"vr5{mj}", name=f"vr5{mj}")
                for mj in range(2)
            ]
            kqp_sb = [
                persist.tile([128, HALF_GRID], F32, tag=f"kqp{mj}", name=f"kqp{mj}")
                for mj in range(2)
            ]
            vp_sb = [
                persist.tile([128, HALF_GRID], F32, tag=f"vp{mj}", name=f"vp{mj}")
                for mj in range(2)
            ]
            key_raw = [
                persist.tile([128, S], F32, tag=f"keyr{mj}", name=f"keyr{mj}")
                for mj in range(2)
            ]
            key_sb = [
                persist.tile([128, S], BF16, tag=f"key{mj}", name=f"key{mj}")
                for mj in range(2)
            ]
            vch_sb = [
                persist.tile([128, S], F32, tag=f"vch{mj}", name=f"vch{mj}")
                for mj in range(2)
            ]
            vchs_sb = [
                persist.tile([128, S], BF16, tag=f"vchs{mj}", name=f"vchs{mj}")
                for mj in range(2)
            ]
            wovT_sb = persist.tile([S, COUT], BF16, tag="wovT", name="wovT")

            ag1_in = dram.tile([2 * 128, HALF_GRID], F32)
            ag1_out = dram.tile([4 * 128, HALF_GRID], F32)
            ag2_in = dram.tile([2 * 128, HALF_GRID], F32)
            ag2_out = dram.tile([4 * 128, HALF_GRID], F32)

            rg = [[0, 1], [2, 3], [4, 5], [6, 7]]

            xall = persist.tile([128, 4, NPIX], BF16, tag="xall", name="xall")
            with (
                tc.tile_pool(name="ps_kq", bufs=2, space="PSUM") as ps_kq,
                tc.tile_pool(name="ps_vt", bufs=2, space="PSUM") as ps_vt,
            ):

                def load_x(g, ng=1):
                    # one strided DMA per 4-row group into the resident x copy
                    nc.sync.dma_start(
                        out=xall[:, :, g * RG * W : (g + 1) * RG * W].rearrange(
                            "p k (a b) -> p k a b", a=RG
                        ),
                        in_=bass.AP(
                            tensor=x_sh[:, :, :].tensor,
                            offset=g * RG * W,
                            ap=[
                                [HL * W, 128],
                                [128 * HL * W, 4],
                                [W, RG],
                                [1, W],
                            ],
                        ),
                    )

                def stage2(rs5, dst, g):
                    """At the conv groups where a 20-row (p6) or 15-row (p8)
                    block completes, fold its 5-col sums into the half grid
                    with one fused column+row reduce per block."""
                    for mj in range(2):
                        if (g + 1) * RG % 20 == 0:
                            rb = (g + 1) * RG // 20 - 1
                            nc.vector.tensor_reduce(
                                dst[mj][:, rb * 6 : (rb + 1) * 6].rearrange(
                                    "p (a b) -> p a b", a=1
                                ),
                                rs5[mj][:, rb * 20 : (rb + 1) * 20, :].rearrange(
                                    "p r (j f) -> p j r f", f=4
                                ),
                                axis=mybir.AxisListType.XY,
                                op=ALU.add,
                            )
                        for rb in range(4):
                            if g * RG < 15 * (rb + 1) <= (g + 1) * RG:
                                nc.vector.tensor_reduce(
                                    dst[mj][
                                        :, 18 + rb * 8 : 18 + (rb + 1) * 8
                                    ].rearrange("p (a b) -> p a b", a=1),
                                    rs5[mj][:, rb * 15 : (rb + 1) * 15, :].rearrange(
                                        "p r (j f) -> p j r f", f=3
                                    ),
                                    axis=mybir.AxisListType.XY,
                                    op=ALU.add,
                                )

                # ============ Phase A1: kq convs + pooling, AG1 =============
                for g in range(NGRP):
                    load_x(g, ng=1)
                    sl = slice(g * RG * W, (g + 1) * RG * W)
                    for mj in range(2):
                        pk = ps_kq.tile([128, RG * W], F32, tag="pkq", name="pkq")
                        for ki in range(4):
                            nc.tensor.matmul(
                                pk,
                                wk_sb[ki][:, mj * 128 : (mj + 1) * 128],
                                xall[:, ki, sl],
                                start=(ki == 0),
                                stop=(ki == 3),
                            )
                        nc.scalar.activation(
                            kq_sb[mj][:, sl],
                            pk,
                            AF.Relu,
                            bias=bn_shift_sb[mj],
                            scale=bn_inv_sb[mj],
                        )
                        # 5-column sums (the gcd of the 20/15 pooling blocks)
                        nc.vector.tensor_reduce(
                            kq_rs5[mj][:, g * RG : (g + 1) * RG, :],
                            kq_sb[mj][:, sl].rearrange(
                                "p (r c f) -> p r c f", r=RG, c=24
                            ),
                            axis=mybir.AxisListType.X,
                            op=ALU.add,
                        )
                    stage2(kq_rs5, kqp_sb, g)
                for mj in range(2):
                    nc.gpsimd.dma_start(
                        out=ag1_in[mj * 128 : (mj + 1) * 128, :], in_=kqp_sb[mj]
                    )
                nc.gpsimd.collective_compute(
                    "AllGather",
                    ALU.bypass,
                    replica_groups=rg,
                    ins=[ag1_in[:, :].opt()],
                    outs=[ag1_out[:, :].opt()],
                )

                def key_fixup():
                    for mj in range(2):
                        dst = key_raw[mj]
                        r0 = mj * 128
                        r1 = (2 + mj) * 128
                        nc.gpsimd.dma_start(
                            out=dst[:, 10:28], in_=ag1_out[r0 : r0 + 128, 0:18]
                        )
                        nc.gpsimd.dma_start(
                            out=dst[:, 28:46], in_=ag1_out[r1 : r1 + 128, 0:18]
                        )
                        nc.gpsimd.dma_start(
                            out=dst[:, 46:78], in_=ag1_out[r0 : r0 + 128, 18:50]
                        )
                        nc.gpsimd.dma_start(
                            out=dst[:, 78:110], in_=ag1_out[r1 : r1 + 128, 18:50]
                        )
                        nc.vector.tensor_reduce(
                            dst[:, 1:10].rearrange("p (a b) -> p a b", a=3),
                            dst[:, 10:46].rearrange(
                                "p (I di J dj) -> p I J di dj", I=3, di=2, J=3
                            ),
                            axis=mybir.AxisListType.XY,
                            op=ALU.add,
                        )
                        nc.vector.tensor_reduce(
                            dst[:, 0:1],
                            dst[:, 10:46],
                            axis=mybir.AxisListType.X,
                            op=ALU.add,
                        )
                        # sums -> means with the 1/sqrt(ck) score scale folded
                        nc.vector.tensor_mul(key_sb[mj], key_raw[mj], kcr_sb)

                # ==== Phase A2: v convs + DVE pooling from PSUM, AG2 =======
                for g in range(NGRP):
                    sl = slice(g * RG * W, (g + 1) * RG * W)
                    for mj in range(2):
                        pv = ps_vt.tile([128, RG * W], F32, tag="pv", name="pv")
                        for ki in range(4):
                            nc.tensor.matmul(
                                pv,
                                wv_sb[ki][:, mj * 128 : (mj + 1) * 128],
                                xall[:, ki, sl],
                                start=(ki == 0),
                                stop=(ki == 3),
                            )
                        nc.vector.tensor_reduce(
                            v_rs5[mj][:, g * RG : (g + 1) * RG, :],
                            pv.rearrange("p (r c f) -> p r c f", r=RG, c=24),
                            axis=mybir.AxisListType.X,
                            op=ALU.add,
                        )
                    stage2(v_rs5, vp_sb, g)
                    if g == 1:
                        # key fixup rides here: AG1 finished during the early
                        # v groups; emitting it before the AG2 staging avoids
                        # head-of-line blocking on the gpsimd DMA queue.
                        key_fixup()
                for mj in range(2):
                    nc.gpsimd.dma_start(
                        out=ag2_in[mj * 128 : (mj + 1) * 128, :], in_=vp_sb[mj]
                    )
                nc.gpsimd.collective_compute(
                    "AllGather",
                    ALU.bypass,
                    replica_groups=rg,
                    ins=[ag2_in[:, :].opt()],
                    outs=[ag2_out[:, :].opt()],
                )

            # ============ Phase C: attention, software-pipelined ============
            with (
                tc.tile_pool(name="ps_sc", bufs=2, space="PSUM") as ps_sc,
                tc.tile_pool(name="ps_cs", bufs=2, space="PSUM") as ps_cs,
                tc.tile_pool(name="ps_wov", bufs=1, space="PSUM") as ps_wov,
                tc.tile_pool(name="ps_out", bufs=3, space="PSUM") as ps_out,
                tc.tile_pool(name="c1p", bufs=4) as sb_c1,
                tc.tile_pool(name="attn_keep", bufs=LAG + 2) as sb_attn,
                tc.tile_pool(name="outp", bufs=3) as sb_out,
            ):
                offs = list(range(0, NPIX, PIXT))
                attn_tiles = {}

                def emit_c1(t):
                    off = offs[t]
                    N = min(PIXT, NPIX - off)
                    psc = ps_sc.tile([S, PIXT], F32, tag="sc", name="sc")[:, :N]
                    for mj in range(2):
                        nc.tensor.matmul(
                            psc,
                            key_sb[mj],
                            kq_sb[mj][:, off : off + N],
                            start=(mj == 0),
                            stop=(mj == 1),
                        )
                    expt = sb_c1.tile([S, PIXT], BF16, tag="exp", name="exp")[:, :N]
                    nc.scalar.activation(expt, psc, AF.Exp)
                    pcs = ps_cs.tile([S, PIXT], F32, tag="cs", name="cs")[:, :N]
                    nc.tensor.matmul(pcs, ones_sb, expt, start=True, stop=True)
                    lncs = sb_c1.tile([S, PIXT], F32, tag="lncs", name="lncs")[:, :N]
                    nc.scalar.activation(lncs, pcs, AF.Ln)
                    rb = sb_c1.tile([S, PIXT], BF16, tag="rb", name="rb")[:, :N]
                    nc.scalar.activation(rb, lncs, AF.Exp, scale=-1.0)
                    attn = sb_attn.tile([S, PIXT], BF16, tag="attn", name="attn")[
                        :, :N
                    ]
                    nc.gpsimd.tensor_mul(attn, expt, rb)
                    attn_tiles[t] = attn

                def emit_c2(t):
                    off = offs[t]
                    N = min(PIXT, NPIX - off)
                    attn = attn_tiles.pop(t)
                    ot = sb_out.tile([128, 4, PIXT], F32, tag="ot", name="ot")[
                        :, :, :N
                    ]
                    for co in range(4):
                        po = ps_out.tile([128, PIXT], F32, tag="out", name="po")[
                            :, :N
                        ]
                        nc.tensor.matmul(
                            po,
                            wovT_sb[:, co * 128 : (co + 1) * 128],
                            attn,
                            start=True,
                            stop=True,
                        )
                        if co % 2 == 0:
                            nc.scalar.activation(
                                ot[:, co, :], po, AF.Identity, bias=bo_sb[co]
                            )
                        else:
                            nc.vector.tensor_scalar_add(ot[:, co, :], po, bo_sb[co])
                    nc.sync.dma_start(
                        out=bass.AP(
                            tensor=out_d[:, :].tensor,
                            offset=off,
                            ap=[[NPIX, 128], [128 * NPIX, 4], [1, N]],
                        ),
                        in_=ot,
                    )

                for t in range(len(offs)):
                    emit_c1(t)
                    if t == LAG - 1:
                        # value fixup + WoV^T (AG2 has landed by now)
                        for mj in range(2):
                            dst = vch_sb[mj]
                            r0 = mj * 128
                            r1 = (2 + mj) * 128
                            nc.gpsimd.dma_start(
                                out=dst[:, 10:28], in_=ag2_out[r0 : r0 + 128, 0:18]
                            )
                            nc.gpsimd.dma_start(
                                out=dst[:, 28:46], in_=ag2_out[r1 : r1 + 128, 0:18]
                            )
                            nc.gpsimd.dma_start(
                                out=dst[:, 46:78], in_=ag2_out[r0 : r0 + 128, 18:50]
                            )
                            nc.gpsimd.dma_start(
                                out=dst[:, 78:110], in_=ag2_out[r1 : r1 + 128, 18:50]
                            )
                            nc.vector.tensor_reduce(
                                dst[:, 1:10].rearrange("p (a b) -> p a b", a=3),
                                dst[:, 10:46].rearrange(
                                    "p (I di J dj) -> p I J di dj", I=3, di=2, J=3
                                ),
                                axis=mybir.AxisListType.XY,
                                op=ALU.add,
                            )
                            nc.vector.tensor_reduce(
                                dst[:, 0:1],
                                dst[:, 10:46],
                                axis=mybir.AxisListType.X,
                                op=ALU.add,
                            )
                            nc.vector.tensor_mul(vchs_sb[mj], vch_sb[mj], vcr_sb)
                        pwov = ps_wov.tile([S, COUT], F32, tag="wov", name="pwov")
                        for mj in range(2):
                            nc.tensor.matmul(
                                pwov,
                                vchs_sb[mj],
                                wo_sb[mj],
                                start=(mj == 0),
                                stop=(mj == 1),
                            )
                        nc.scalar.activation(wovT_sb, pwov, AF.Copy)
                    if t >= LAG:
                        emit_c2(t - LAG)
                for t in range(len(offs) - LAG, len(offs)):
                    emit_c2(t)
    _split_excess_waits(nc)
    return nc


_CACHE = {}


def _get_nc():
    if "nc" not in _CACHE:
        _CACHE["nc"] = build_nc()
    return _CACHE["nc"]


def _make_masks(h):
    """Per-row pooling masks for local rows of half h: mask[c, r, s] = 1 if
    pixel column c belongs to region s for global row 60*h + r."""
    m = np.zeros((120, HL, S), np.float32)
    for r in range(HL):
        g = h * HL + r
        c = np.arange(120)
        m[:, r, 0] = 1.0
        m[c, r, 1 + (g // 40) * 3 + c // 40] = 1.0
        m[c, r, 10 + (g // 20) * 6 + c // 20] = 1.0
        m[c, r, 46 + (g // 15) * 8 + c // 15] = 1.0
    return m


def kernel(x, Wk, bk, gamma, beta, mean, var, Wv, bv, Wo, bo):
    import ml_dtypes

    from concourse.bass_utils import run_bass_kernel_spmd

    bf16 = ml_dtypes.bfloat16
    x = np.asarray(x, np.float32)
    Wk = np.asarray(Wk, np.float32)
    bk = np.asarray(bk, np.float32)
    gamma = np.asarray(gamma, np.float32)
    beta = np.asarray(beta, np.float32)
    mean = np.asarray(mean, np.float32)
    var = np.asarray(var, np.float32)
    Wv = np.asarray(Wv, np.float32)
    bv = np.asarray(bv, np.float32)
    Wo = np.asarray(Wo, np.float32)
    bo = np.asarray(bo, np.float32)

    inv = gamma / np.sqrt(var + EPS)
    # reference: kq = (Wk x + bk), then BN: kq*inv + (beta - mean*inv)
    shift = beta - mean * inv + bk * inv
    # value = pooled_mean + bv and out = Wo @ value ... : Wo @ bv joins bias
    bo_eff = bo + Wo @ bv

    counts = np.concatenate(
        [
            np.full(1, H * W, np.float32),
            np.full(9, (H // 3) * (W // 3), np.float32),
            np.full(36, (H // 6) * (W // 6), np.float32),
            np.full(64, (H // 8) * (W // 8), np.float32),
        ]
    )
    key_crecip = (
        np.broadcast_to((1.0 / counts)[None, :] * (CK**-0.5), (128, S))
        .astype(np.float32)
        .copy()
    )
    val_crecip = (
        np.broadcast_to((1.0 / counts)[None, :], (128, S)).astype(np.float32).copy()
    )

    common = {
        "wkT": np.ascontiguousarray(Wk.T).astype(bf16),
        "wvT": np.ascontiguousarray(Wv.T).astype(bf16),
        "woT": np.ascontiguousarray(Wo.T).astype(bf16),
        "ones_mat": np.ones((S, S), bf16),
        "bn_inv": inv[:, None].copy(),
        "bn_shift": shift[:, None].copy(),
        "bo_eff": bo_eff[:, None].copy(),
        "key_crecip": key_crecip,
        "val_crecip": val_crecip,
    }
    in_maps = []
    for c in range(N_CORES):
        b, h = c // 2, c % 2
        m = dict(common)
        m["x_sh"] = np.ascontiguousarray(x[b, :, h * HL : (h + 1) * HL, :]).astype(
            bf16
        )
        in_maps.append(m)

    nc = _get_nc()
    _CACHE["last_in_maps"] = in_maps
    res = run_bass_kernel_spmd(nc, in_maps, core_ids=list(range(N_CORES)))
    out = np.empty((B, COUT, H, W), np.float32)
    for c in range(N_CORES):
        b, h = c // 2, c % 2
        out[b, :, h * HL : (h + 1) * HL, :] = res.results[c]["out"].reshape(
            COUT, HL, W
        )
    return out
